# revision 25
# baseline (speedup 1.0000x reference)
"""Trainium2 Bass kernel for an AttentionBlock:
GroupNorm(8 groups) -> 1x1 conv q/k/v -> multi-head attention (4 heads)
-> 1x1 conv proj -> residual add.

Shapes (hardcoded): x [4, 256, 64, 64]; L = 64*64 = 4096; head dim 64.

Sharding: 8 cores = (batch, query-half). Each core computes the full
GroupNorm + K/V for its batch, and attention + projection + residual for
its half (2048) of the query positions. Host permutes each batch's pixel
columns so a core's query half is always columns 0:2048, so all 8 cores
run one SPMD program. No collectives; host concatenates.

v3 design (over the v2 bf16 baseline):
- S matmuls contract only K=64 (head dim), so the two heads of a channel
  tile are issued as an adjacent quartet at PE row groups 0-1 / 2-3
  (tile_position auto-derived from base partition 0 / 64): the PE runs
  the two heads' S matmuls concurrently, halving S cost.
- exp is split per head: head-even on ScalarE (exact exp -> bf16),
  head-odd on VectorE (Schraudolph: bf16 bit pattern via one
  tensor_scalar u16 = floor(a*S + b)), so both exps of a pair run
  concurrently and the S quartet of the next pair issues sooner.
- GroupNorm scale/shift is folded into the QKV weights on device
  (W' = W*diag(s), b' = W^T t + b via 12 tiny PE matvecs), so no
  normalized-activation pass exists; QKV contracts a bf16 copy of x
  made by ScalarE Copy during the x DMA window.
- A warm-up stream of dummy N=512 matmuls keeps the PE HAM clock gate
  at 8/8 (2.4 GHz) through the DMA/stats prologue; without it the PE
  runs at 1.2 GHz until ~56us in.
- softmax denominator Z via a 65th ones-column in V^T so the PV matmul
  accumulates Z as row 64; softmax max-subtraction skipped (logits in
  [-8, 8] for normalized inputs).
"""

import numpy as np

B, C, H, W = 4, 256, 64, 64
NH, G, EPS = 4, 8, 1e-5
L = H * W            # 4096
DH = L // 2          # query positions per core
HD = C // NH         # 64
P = 128              # SBUF partitions
CT = C // P          # channel tiles (2)
LC = L // 512        # 8 key-dim 512-chunks
DBLK = DH // 512     # 4 query-dim 512-blocks
ECH = L // P         # 32 key-dim 128-chunks
NPAIR = ECH // 2     # 16 key-dim 256-pairs
NGRP = ECH // 4      # 8 vT groups (4 chunks each = 2 pairs)
SCALE = float(HD) ** -0.5
NCORES = 8

LN2 = float(np.log(2.0))
DVE_A = 128.0 / LN2 * SCALE    # bf16 bits per raw logit (Schraudolph)
DVE_B = 128.0 * 127.0 + 0.5 - 5.5
WARMN = 32                     # HAM warm-up matmuls covering the prologue

_CACHE = {}


def _build_nc():
    import concourse.bacc as bacc
    import concourse.bass as bass
    import concourse.mybir as mybir
    import concourse.tile as tile
    from concourse.masks import make_identity
    from contextlib import ExitStack

    f32 = mybir.dt.float32
    f16 = mybir.dt.float16
    bf16 = mybir.dt.bfloat16
    f32r = mybir.dt.float32r
    u8 = mybir.dt.uint8
    u16 = mybir.dt.uint16
    AX = mybir.AxisListType
    OP = mybir.AluOpType
    ACT = mybir.ActivationFunctionType

    def r(ap):
        return ap.bitcast(f32r)

    def mktile(pool, shape, tag, dtype=None):
        return pool.tile(shape, dtype or f32, name=tag, tag=tag)

    nc = bacc.Bacc(trn_type="TRN2", target_bir_lowering=False, num_devices=NCORES)

    x_ext = nc.declare_dram_parameter("x", [C, L], f32, isOutput=False)
    wq_ext = nc.declare_dram_parameter("wqt", [C, C], bf16, isOutput=False)
    wk_ext = nc.declare_dram_parameter("wkt", [C, C], bf16, isOutput=False)
    wv_ext = nc.declare_dram_parameter("wvt", [C, C], bf16, isOutput=False)
    wp_ext = nc.declare_dram_parameter("wpt", [C, C], bf16, isOutput=False)
    # smalls[:, 0:6] = bq,bk,bv,bp,gnw,gnb; [:, 6:14] = group indicator
    smalls_ext = nc.declare_dram_parameter("smalls", [C, 16], f32, isOutput=False)
    indT_ext = nc.declare_dram_parameter("indT", [G, C], f32, isOutput=False)
    out_ext = nc.declare_dram_parameter("out", [C, DH], f32, isOutput=True)

    with tile.TileContext(nc) as tc, ExitStack() as top:
        # ---- pool A: kernel-long tiles -------------------------------
        pa = top.enter_context(tc.tile_pool(name="pa", bufs=1))
        warm16 = mktile(pa, [P, 512], "warm16", dtype=bf16)
        nc.vector.memset(warm16[:], 0.001)
        # allocate kernel-long tiles now; their DMAs are emitted AFTER the
        # x chunks so x owns the head of both DMA queues
        smalls = [mktile(pa, [P, 16], f"smalls{ct}") for ct in range(CT)]
        indTt = mktile(pa, [G, C], "indTt")
        indt_t = [mktile(pa, [P, 16], f"ind{ct}") for ct in range(CT)]
        bias = {}
        for j, nm in enumerate(("bq", "bk", "bv", "bp", "gnw", "gnb")):
            bias[nm] = [smalls[ct][:, j:j + 1] for ct in range(CT)]
        indt = [t[:, 6:6 + G] for t in indt_t]
        ones_t = mktile(pa, [P, 2 * ECH], "ones_t")
        nc.vector.memset(ones_t[:], 1.0)
        ones_src = ones_t[:]
        wts = {}
        for nm, ext in (("wqt", wq_ext), ("wkt", wk_ext), ("wvt", wv_ext)):
            wts[nm] = []
            for ct in range(CT):
                t = mktile(pa, [P, C], f"{nm}{ct}", dtype=bf16)
                wts[nm].append(t)
        wts_dma = [(nm, ext) for nm, ext in
                   (("wqt", wq_ext), ("wkt", wk_ext), ("wvt", wv_ext))]
        wpth = [mktile(pa, [HD, C], f"wpt{h}", dtype=bf16) for h in range(NH)]
        ident = mktile(pa, [P, P], "ident")
        make_identity(nc, ident[:])
        ident16 = mktile(pa, [P, P], "ident16", dtype=bf16)
        nc.vector.tensor_copy(out=ident16[:], in_=ident[:])
        bones16 = mktile(pa, [P, 64], "bones16", dtype=bf16)
        nc.vector.memset(bones16[:], 1.0)
        # attention output, one tile per head [64, DH] at base partition 0
        ao = [mktile(pa, [HD, DH], f"ao{h}", dtype=bf16) for h in range(NH)]

        # ---- pool B: bf16 copy of x (QKV moving operand) -------------
        pb = top.enter_context(tc.tile_pool(name="pb", bufs=1))
        x16 = [mktile(pb, [P, L], f"x16_{ct}", dtype=bf16) for ct in range(CT)]

        # ---- warm-up: HAM stream + activation tables -----------------
        with ExitStack() as pw:
            pwp = pw.enter_context(tc.tile_pool(name="pwp", bufs=1, space="PSUM"))
            pws = pw.enter_context(tc.tile_pool(name="pws", bufs=1))
            wps = mktile(pwp, [P, 512], "wps")
            for i in range(WARMN):
                nc.tensor.matmul(wps[:], warm16[:, 0:P], warm16[:],
                                 start=True, stop=True)
            wsc = mktile(pws, [P, 2], "wsc")
            nc.scalar.activation(wsc[:, 0:1], warm16[:, 0:1].bitcast(bf16), ACT.Exp)
            nc.scalar.activation(wsc[:, 1:2], wsc[:, 0:1], ACT.Ln)

        # ---- GroupNorm stats + weight fold ---------------------------
        with ExitStack() as ph:
            px = ph.enter_context(tc.tile_pool(name="px", bufs=1))
            pgs = ph.enter_context(tc.tile_pool(name="pgs", bufs=1))
            pgp = ph.enter_context(tc.tile_pool(name="pgp", bufs=1, space="PSUM"))
            xs = [mktile(px, [P, L], f"x{ct}") for ct in range(CT)]
            # x streams on two DMA queues: ct0 via sync, ct1 via gpsimd
            for ct in range(CT):
                eng = nc.sync if ct == 0 else nc.gpsimd
                for lc in range(LC):
                    sl = slice(lc * 512, (lc + 1) * 512)
                    eng.dma_start(out=r(xs[ct][:, sl]),
                                  in_=r(x_ext[ct * P:(ct + 1) * P, sl]))
                    # bf16 copy for the QKV contraction, on ScalarE
                    nc.scalar.activation(x16[ct][:, sl], xs[ct][:, sl], ACT.Copy)
            # everything else follows x on the two queues
            for ct in range(CT):
                nc.sync.dma_start(out=smalls[ct][:],
                                  in_=smalls_ext[ct * P:(ct + 1) * P, :])
                nc.sync.dma_start(
                    out=r(indt_t[ct][:]),
                    in_=r(smalls_ext[ct * P:(ct + 1) * P, :]))
            nc.sync.dma_start(out=indTt[:], in_=indT_ext[:])
            for nm, ext in wts_dma:
                for ct in range(CT):
                    nc.gpsimd.dma_start(out=wts[nm][ct][:],
                                        in_=ext[ct * P:(ct + 1) * P, :])
            for h in range(NH):
                nc.sync.dma_start(out=wpth[h][:],
                                  in_=wp_ext[h * HD:(h + 1) * HD, :])
            # per-partition stats via bn_stats/bn_aggr, then a tiny PE
            # matmul with the group indicator to combine across partitions
            pm = []
            for ct in range(CT):
                st6 = mktile(pgs, [P, LC * 6], f"st6_{ct}")
                for lc in range(LC):
                    nc.vector.bn_stats(st6[:, lc * 6:(lc + 1) * 6],
                                       xs[ct][:, lc * 512:(lc + 1) * 512])
                mv = mktile(pgs, [P, 2], f"mv{ct}")
                nc.vector.bn_aggr(mv[:], st6[:].rearrange("p (a b) -> p a b", b=6))
                m2 = mktile(pgs, [P, 2], f"m2_{ct}")   # [mean, var + mean^2]
                nc.vector.tensor_copy(out=r(m2[:, 0:1]), in_=mv[:, 0:1])
                nc.vector.scalar_tensor_tensor(
                    out=r(m2[:, 1:2]), in0=mv[:, 0:1], scalar=0.0,
                    in1=mv[:, 0:1], op0=OP.add, op1=OP.mult)
                nc.vector.tensor_add(r(m2[:, 1:2]), r(m2[:, 1:2]).bitcast(f32),
                                     mv[:, 1:2])
                pm.append(m2)
            gsum = mktile(pgp, [G, 2], "gsum")
            for ct in range(CT):
                nc.tensor.matmul(gsum[:], r(indt[ct]), r(pm[ct][:]),
                                 start=(ct == 0), stop=(ct == CT - 1))
            inv_np = 1.0 / float(P // G * CT)
            mrs = mktile(pgs, [G, 2], "mrs")      # col0 = mean, col1 = rstd
            var = mktile(pgs, [G, 1], "var")
            sqv = mktile(pgs, [G, 1], "sqv")
            nc.vector.tensor_scalar_mul(mrs[:, 0:1], gsum[:, 0:1], inv_np)
            nc.vector.tensor_scalar_mul(var[:], gsum[:, 1:2], inv_np)
            nc.vector.tensor_mul(sqv[:], mrs[:, 0:1], mrs[:, 0:1])
            nc.vector.tensor_sub(var[:], var[:], sqv[:])
            eps_t = mktile(pgs, [G, 1], "eps")
            nc.vector.memset(eps_t[:], EPS)
            nc.scalar.activation(sqv[:], var[:], ACT.Ln, bias=eps_t[:])
            nc.scalar.activation(mrs[:, 1:2], sqv[:], ACT.Exp, scale=-0.5)
            # broadcast group stats to channels via PE: bc[c, :] = mrs[g(c), :]
            sts = []
            for ct in range(CT):
                bc_ps = mktile(pgp, [P, 2], f"bcps{ct}")
                nc.tensor.matmul(bc_ps[:], indTt[:, ct * P:(ct + 1) * P],
                                 mrs[:], start=True, stop=True)
                s_t = mktile(pgs, [P, 1], f"s{ct}")
                t_t = mktile(pgs, [P, 1], f"t{ct}")
                nc.vector.tensor_mul(s_t[:], bc_ps[:, 1:2], bias["gnw"][ct])
                nc.vector.tensor_mul(t_t[:], bc_ps[:, 0:1], s_t[:])
                nc.vector.tensor_sub(t_t[:], bias["gnb"][ct], t_t[:])
                sts.append((s_t, t_t))
            # fold GN into QKV: b' = W^T t + b (PE matvecs on the original
            # weights), then W' = W * diag(s) in place
            t16 = []
            for ct in range(CT):
                t = mktile(pgs, [P, 1], f"t16_{ct}", dtype=bf16)
                nc.vector.tensor_copy(out=t[:], in_=sts[ct][1][:])
                t16.append(t)
            for wnm, bnm in (("wqt", "bq"), ("wkt", "bk"), ("wvt", "bv")):
                for oct in range(CT):
                    osl = slice(oct * P, (oct + 1) * P)
                    fps = mktile(pgp, [P, 1], "fps")
                    for ct in range(CT):
                        nc.tensor.matmul(fps[:], wts[wnm][ct][:, osl],
                                         t16[ct][:],
                                         start=(ct == 0), stop=(ct == CT - 1))
                    nc.vector.tensor_add(bias[bnm][oct],
                                         bias[bnm][oct], fps[:])
            for wnm in ("wqt", "wkt", "wvt"):
                for ct in range(CT):
                    nc.vector.tensor_scalar_mul(wts[wnm][ct][:],
                                                wts[wnm][ct][:], sts[ct][0][:])

        # ---- pool C: q/k/VT (live through attention) -----------------
        pc = top.enter_context(tc.tile_pool(name="pc", bufs=1))
        qt = [mktile(pc, [P, DH], f"q{p}", dtype=bf16) for p in range(CT)]
        kt = [mktile(pc, [P, L], f"k{p}", dtype=bf16) for p in range(CT)]
        vt = [mktile(pc, [P, 65 * ECH], f"vt{h}", dtype=bf16) for h in range(NH)]

        # ---- QKV + V^T, as chunk closures ---------------------------
        vpair = [mktile(pc, [P, L], f"v{p}", dtype=bf16) for p in range(CT)]

        def qkv_chunks(p, paux):
            """Returns (early, deferred): minimal set needed for the first
            attention pairs, and the rest ordered by need."""
            osl = slice(p * P, (p + 1) * P)

            def qkv_go(wnm, bnm, dst, cchunk):
                def go():
                    sl = slice(cchunk * 512, (cchunk + 1) * 512)
                    ps = paux.tile([P, 512], f32, name="aux", tag="aux")
                    for ct in range(CT):
                        nc.tensor.matmul(ps[:], wts[wnm][ct][:, osl],
                                         x16[ct][:, sl],
                                         start=(ct == 0), stop=(ct == CT - 1))
                    nc.vector.tensor_scalar_add(dst[:, sl], ps[:], bias[bnm][p])
                return go

            def vt_go(ecg):
                def go():
                    pst = (paux.tile([P, 512], f32, name="aux", tag="aux")[:]
                           .bitcast(bf16)[:, 0:512])
                    for j in range(4):
                        esl = slice((ecg * 4 + j) * P, (ecg * 4 + j + 1) * P)
                        nc.tensor.transpose(pst[:, j * P:(j + 1) * P],
                                            vpair[p][:, esl], ident16[:])
                    for h01 in range(2):
                        head = 2 * p + h01
                        outap = (vt[head][:, ecg * 260:(ecg + 1) * 260]
                                 .rearrange("p (a b) -> p a b", b=65)[:, :, 0:64])
                        inap = (pst[:].rearrange("p (a b) -> p a b", b=P)
                                [:, :, h01 * 64:(h01 + 1) * 64])
                        nc.vector.tensor_copy(out=outap, in_=inap)
                return go

            def ones_go(h01):
                def go():
                    h = 2 * p + h01
                    ones_ap = (vt[h][:].rearrange("p (a b) -> p a b", b=65)
                               [:, :, 64:65])
                    nc.vector.tensor_copy(
                        out=ones_ap,
                        in_=ones_src[:, p * ECH:(p + 1) * ECH]
                        .rearrange("p a -> p a ()"))
                return go

            qw = [qkv_go("wqt", "bq", qt[p], c) for c in range(DBLK)]
            kw = [qkv_go("wkt", "bk", kt[p], c) for c in range(LC)]
            vw = [qkv_go("wvt", "bv", vpair[p], c) for c in range(LC)]
            tw = [vt_go(g) for g in range(NGRP)]
            ow = [ones_go(h01) for h01 in range(2)]
            early = [qw[0], kw[0], kw[1], vw[0], tw[0]] + ow
            deferred = [vw[1], tw[1]]
            for c in range(2, LC):
                deferred += [kw[c], vw[c], tw[c]]
            deferred += qw[1:]
            return early, deferred

        # ---- attention + per-db projection --------------------------
        with ExitStack() as ph:
            pe_s = ph.enter_context(tc.tile_pool(name="pes", bufs=3))
            pf = ph.enter_context(tc.tile_pool(name="pf", bufs=1))
            pfs = ph.enter_context(tc.tile_pool(name="pfs", bufs=3))
            ppt = ph.enter_context(tc.tile_pool(name="ppt", bufs=4))
            psp = ph.enter_context(tc.tile_pool(name="psp", bufs=1, space="PSUM"))
            pap = ph.enter_context(tc.tile_pool(name="pap", bufs=3, space="PSUM"))
            paux = ph.enter_context(tc.tile_pool(name="paux", bufs=1, space="PSUM"))
            xres = []
            for ct in range(CT):
                t = mktile(pf, [P, DH], f"xr{ct}")
                nc.sync.dma_start(out=t[:], in_=x_ext[ct * P:(ct + 1) * P, 0:DH])
                xres.append(t)
            e0, d0 = qkv_chunks(0, paux)
            e1, d1 = qkv_chunks(1, paux)
            for fn in e0 + e1[-2:]:
                fn()
            deferred = d0 + e1[:-2] + d1

            def epilogue_head(p, db, acc, h01):
                dsl = slice(db * 512, (db + 1) * 512)
                head = 2 * p + h01
                zc = mktile(pe_s, [65, 512], "zc", dtype=bf16)
                nc.scalar.activation(zc[:], acc[h01][0:65, :], ACT.Copy)
                zb_ps = paux.tile([P, 512], f32, name="aux", tag="aux")
                nc.tensor.matmul(zb_ps[0:64, :], bones16[64:65, 0:64],
                                 zc[64:65, :], start=True, stop=True)
                zb = mktile(pe_s, [64, 512], "zb")
                with nc.allow_low_precision("softmax denom, well conditioned"):
                    nc.vector.reciprocal_approx_fast(zb[:], zb_ps[0:64, :])
                nc.vector.tensor_mul(ao[head][:, dsl],
                                     zc[0:64, :], zb[:])

            def proj_ot(db, ot):
                dsl = slice(db * 512, (db + 1) * 512)
                osl = slice(ot * P, (ot + 1) * P)
                ps = paux.tile([P, 512], f32, name="aux", tag="aux")
                for h in range(NH):
                    nc.tensor.matmul(ps[:], wpth[h][:, osl],
                                     ao[h][:, dsl],
                                     start=(h == 0), stop=(h == NH - 1))
                osb = mktile(pfs, [P, 512], "osb")
                nc.vector.scalar_tensor_tensor(
                    out=osb[:], in0=ps[:], scalar=bias["bp"][ot],
                    in1=xres[ot][:, dsl], op0=OP.add, op1=OP.add)
                nc.sync.dma_start(out=out_ext[osl, dsl], in_=osb[:])

            def s_quartet(p, db, pair, slot):
                # 4 S matmuls issued adjacently into ONE [128,2048] psum
                # tile (h0 at cols 0:1024, h1 at 1024:2048); head0 at PE
                # rows 0-63, head1 at rows 64-127 (row groups auto-derived
                # from base partition).  The shared tile makes every
                # quartet matmul WAR-wait on BOTH engines' exps of the
                # previous pair, so the four matmuls become ready together
                # and the two row groups overlap on the PE.
                dsl = slice(db * 512, (db + 1) * 512)
                for kti in range(2):
                    ec = 2 * pair + kti
                    esl = slice(ec * P, (ec + 1) * P)
                    for h01 in range(2):
                        hsl = slice(h01 * 64, (h01 + 1) * 64)
                        o = h01 * 1024 + kti * 512
                        nc.tensor.matmul(
                            slot[:, o:o + 512],
                            kt[p][hsl, esl], qt[p][hsl, dsl],
                            start=True, stop=True)

            pending = []
            for db in range(DBLK):
                for p in range(CT):
                    acc = [mktile(pap, [65, 512], "acc") for _ in range(2)]
                    slot = mktile(psp, [P, 2048], "sps")
                    s_quartet(p, db, 0, slot)
                    for pair in range(NPAIR):
                        pts = []
                        for h01 in range(2):
                            if pair >= 1 or db + p > 0:
                                budget = 2 if len(deferred) > 20 else 1
                                for _ in range(budget):
                                    if deferred:
                                        deferred.pop(0)()
                                    elif pending:
                                        pending.pop(0)()
                                        break
                            pt = mktile(ppt, [P, 1024], "pt", dtype=u16)
                            ssl = slice(h01 * 1024, (h01 + 1) * 1024)
                            if h01 == 1:
                                nc.vector.tensor_scalar(
                                    pt[:], slot[:, ssl], DVE_A, DVE_B,
                                    op0=OP.mult, op1=OP.add)
                            else:
                                nc.scalar.activation(pt[:].bitcast(bf16),
                                                     slot[:, ssl], ACT.Exp,
                                                     scale=SCALE)
                            pts.append(pt)
                        if pair + 1 < NPAIR:
                            slot = mktile(psp, [P, 2048], "sps")
                            s_quartet(p, db, pair + 1, slot)
                        for h01 in range(2):
                            head = 2 * p + h01
                            pt16 = pts[h01][:].bitcast(bf16)
                            for kti in range(2):
                                ec = 2 * pair + kti
                                nc.tensor.matmul(
                                    acc[h01][:],
                                    vt[head][:, ec * 65:(ec + 1) * 65],
                                    pt16[:, kti * 512:(kti + 1) * 512],
                                    start=(pair + kti == 0),
                                    stop=(pair == NPAIR - 1 and kti == 1))
                    for h01 in range(2):
                        pending.append(
                            lambda p=p, db=db, acc=acc, h01=h01:
                            epilogue_head(p, db, acc, h01))
                    if p == CT - 1:
                        for ot in range(CT):
                            pending.append(lambda db=db, ot=ot: proj_ot(db, ot))
            for fn in deferred + pending:
                fn()

    nc.compile()
    return nc


def _in_maps(inputs):
    import ml_dtypes
    bf = ml_dtypes.bfloat16
    x = np.asarray(inputs["x"], dtype=np.float32)
    gnw = np.ascontiguousarray(np.asarray(inputs["gn_w"], np.float32).reshape(C, 1))
    gnb = np.ascontiguousarray(np.asarray(inputs["gn_b"], np.float32).reshape(C, 1))
    wqt = np.ascontiguousarray(np.asarray(inputs["wq"], np.float32).T.astype(bf))
    wkt = np.ascontiguousarray(np.asarray(inputs["wk"], np.float32).T.astype(bf))
    wvt = np.ascontiguousarray(np.asarray(inputs["wv"], np.float32).T.astype(bf))
    wpt = np.ascontiguousarray(np.asarray(inputs["wp"], np.float32).T.astype(bf))
    ind = np.zeros((C, G), np.float32)
    ind[np.arange(C), np.arange(C) // (C // G)] = 1.0
    indT = np.ascontiguousarray(ind.T)
    smalls = np.zeros((C, 16), np.float32)
    for j, nm in enumerate(("bq", "bk", "bv", "bp")):
        smalls[:, j] = np.asarray(inputs[nm], np.float32).reshape(C)
    smalls[:, 4] = gnw.reshape(C)
    smalls[:, 5] = gnb.reshape(C)
    smalls[:, 6:6 + G] = ind
    common = dict(wqt=wqt, wkt=wkt, wvt=wvt, wpt=wpt, smalls=smalls,
                  indT=indT)
    maps = []
    for core in range(NCORES):
        b, half = core // 2, core % 2
        xb = np.ascontiguousarray(x[b].reshape(C, L))
        if half == 1:
            xb = np.ascontiguousarray(
                np.concatenate([xb[:, DH:], xb[:, :DH]], axis=1))
        maps.append(dict(common, x=xb))
    return maps


def kernel(**inputs) -> np.ndarray:
    from concourse.bass_utils import run_bass_kernel_spmd

    if "nc" not in _CACHE:
        _CACHE["nc"] = _build_nc()
    nc = _CACHE["nc"]
    res = run_bass_kernel_spmd(nc, _in_maps(inputs), core_ids=list(range(NCORES)))
    out = np.empty((B, C, L), np.float32)
    for core in range(NCORES):
        b, half = core // 2, core % 2
        out[b][:, half * DH:(half + 1) * DH] = res.results[core]["out"]
    return out.reshape(B, C, L).reshape(B, C, H, W)


# revision 26
# speedup vs baseline: 1.3648x; 1.3648x over previous
"""Trainium2 Bass kernel for an AttentionBlock:
GroupNorm(8 groups) -> 1x1 conv q/k/v -> multi-head attention (4 heads)
-> 1x1 conv proj -> residual add.

Shapes (hardcoded): x [4, 256, 64, 64]; L = 64*64 = 4096; head dim 64.

Sharding: 8 cores = (batch, query-half). Each core computes the full
GroupNorm + K/V for its batch, and attention + projection + residual for
its half (2048) of the query positions. Host permutes each batch's pixel
columns so a core's query half is always columns 0:2048, so all 8 cores
run one SPMD program. No collectives; host concatenates.

v3 design (over the v2 bf16 baseline):
- S matmuls contract only K=64 (head dim), so the two heads of a channel
  tile are issued as an adjacent quartet at PE row groups 0-1 / 2-3
  (tile_position auto-derived from base partition 0 / 64): the PE runs
  the two heads' S matmuls concurrently, halving S cost.
- exp is split per head: head-even on ScalarE (exact exp -> bf16),
  head-odd on VectorE (Schraudolph: bf16 bit pattern via one
  tensor_scalar u16 = floor(a*S + b)), so both exps of a pair run
  concurrently and the S quartet of the next pair issues sooner.
- GroupNorm scale/shift is folded into the QKV weights on device
  (W' = W*diag(s), b' = W^T t + b via 12 tiny PE matvecs), so no
  normalized-activation pass exists; QKV contracts a bf16 copy of x
  made by ScalarE Copy during the x DMA window.
- A warm-up stream of dummy N=512 matmuls keeps the PE HAM clock gate
  at 8/8 (2.4 GHz) through the DMA/stats prologue; without it the PE
  runs at 1.2 GHz until ~56us in.
- softmax denominator Z via a 65th ones-column in V^T so the PV matmul
  accumulates Z as row 64; softmax max-subtraction skipped (logits in
  [-8, 8] for normalized inputs).
"""

import numpy as np

B, C, H, W = 4, 256, 64, 64
NH, G, EPS = 4, 8, 1e-5
L = H * W            # 4096
DH = L // 2          # query positions per core
HD = C // NH         # 64
P = 128              # SBUF partitions
CT = C // P          # channel tiles (2)
LC = L // 512        # 8 key-dim 512-chunks
DBLK = DH // 512     # 4 query-dim 512-blocks
ECH = L // P         # 32 key-dim 128-chunks
NPAIR = ECH // 2     # 16 key-dim 256-pairs
NGRP = ECH // 4      # 8 vT groups (4 chunks each = 2 pairs)
SCALE = float(HD) ** -0.5
NCORES = 8

LN2 = float(np.log(2.0))
DVE_A = 128.0 / LN2 * SCALE    # bf16 bits per raw logit (Schraudolph)
DVE_B = 128.0 * 127.0 + 0.5 - 5.5
WARMN = 32                     # HAM warm-up matmuls covering the prologue

_CACHE = {}


def _build_nc():
    import concourse.bacc as bacc
    import concourse.bass as bass
    import concourse.mybir as mybir
    import concourse.tile as tile
    from concourse.masks import make_identity
    from contextlib import ExitStack

    f32 = mybir.dt.float32
    f16 = mybir.dt.float16
    bf16 = mybir.dt.bfloat16
    f32r = mybir.dt.float32r
    u8 = mybir.dt.uint8
    u16 = mybir.dt.uint16
    AX = mybir.AxisListType
    OP = mybir.AluOpType
    ACT = mybir.ActivationFunctionType

    def r(ap):
        return ap.bitcast(f32r)

    def mktile(pool, shape, tag, dtype=None):
        return pool.tile(shape, dtype or f32, name=tag, tag=tag)

    nc = bacc.Bacc(trn_type="TRN2", target_bir_lowering=False, num_devices=NCORES)

    x_ext = nc.declare_dram_parameter("x", [C, L], f32, isOutput=False)
    wq_ext = nc.declare_dram_parameter("wqt", [C, C], bf16, isOutput=False)
    wk_ext = nc.declare_dram_parameter("wkt", [C, C], bf16, isOutput=False)
    wv_ext = nc.declare_dram_parameter("wvt", [C, C], bf16, isOutput=False)
    wp_ext = nc.declare_dram_parameter("wpt", [C, C], bf16, isOutput=False)
    # smalls[:, 0:6] = bq,bk,bv,bp,gnw,gnb; [:, 6:14] = group indicator
    smalls_ext = nc.declare_dram_parameter("smalls", [C, 16], f32, isOutput=False)
    indT_ext = nc.declare_dram_parameter("indT", [G, C], f32, isOutput=False)
    out_ext = nc.declare_dram_parameter("out", [C, DH], f32, isOutput=True)

    with tile.TileContext(nc) as tc, ExitStack() as top:
        # ---- pool A: kernel-long tiles -------------------------------
        pa = top.enter_context(tc.tile_pool(name="pa", bufs=1))
        warm16 = mktile(pa, [P, 512], "warm16", dtype=bf16)
        nc.vector.memset(warm16[:], 0.001)
        # allocate kernel-long tiles now; their DMAs are emitted AFTER the
        # x chunks so x owns the head of both DMA queues
        smalls = [mktile(pa, [P, 16], f"smalls{ct}") for ct in range(CT)]
        indTt = mktile(pa, [G, C], "indTt")
        indt_t = [mktile(pa, [P, 16], f"ind{ct}") for ct in range(CT)]
        bias = {}
        for j, nm in enumerate(("bq", "bk", "bv", "bp", "gnw", "gnb")):
            bias[nm] = [smalls[ct][:, j:j + 1] for ct in range(CT)]
        indt = [t[:, 6:6 + G] for t in indt_t]
        ones_t = mktile(pa, [P, 2 * ECH], "ones_t")
        nc.vector.memset(ones_t[:], 1.0)
        ones_src = ones_t[:]
        wts = {}
        for nm, ext in (("wqt", wq_ext), ("wkt", wk_ext), ("wvt", wv_ext)):
            wts[nm] = []
            for ct in range(CT):
                t = mktile(pa, [P, C], f"{nm}{ct}", dtype=bf16)
                wts[nm].append(t)
        wts_dma = [(nm, ext) for nm, ext in
                   (("wqt", wq_ext), ("wkt", wk_ext), ("wvt", wv_ext))]
        wpth = [mktile(pa, [HD, C], f"wpt{h}", dtype=bf16) for h in range(NH)]
        ident = mktile(pa, [P, P], "ident")
        make_identity(nc, ident[:])
        ident16 = mktile(pa, [P, P], "ident16", dtype=bf16)
        nc.vector.tensor_copy(out=ident16[:], in_=ident[:])
        bones16 = mktile(pa, [P, 64], "bones16", dtype=bf16)
        nc.vector.memset(bones16[:], 1.0)
        # attention output, one tile per head [64, DH] at base partition 0
        ao = [mktile(pa, [HD, DH], f"ao{h}", dtype=bf16) for h in range(NH)]

        # ---- pool B: bf16 copy of x (QKV moving operand) -------------
        pb = top.enter_context(tc.tile_pool(name="pb", bufs=1))
        x16 = [mktile(pb, [P, L], f"x16_{ct}", dtype=bf16) for ct in range(CT)]

        # ---- warm-up: HAM stream + activation tables -----------------
        with ExitStack() as pw:
            pwp = pw.enter_context(tc.tile_pool(name="pwp", bufs=1, space="PSUM"))
            pws = pw.enter_context(tc.tile_pool(name="pws", bufs=1))
            wps = mktile(pwp, [P, 512], "wps")
            for i in range(WARMN):
                nc.tensor.matmul(wps[:], warm16[:, 0:P], warm16[:],
                                 start=True, stop=True)
            wsc = mktile(pws, [P, 2], "wsc")
            nc.scalar.activation(wsc[:, 0:1], warm16[:, 0:1].bitcast(bf16), ACT.Exp)
            nc.scalar.activation(wsc[:, 1:2], wsc[:, 0:1], ACT.Ln)

        # ---- GroupNorm stats + weight fold ---------------------------
        with ExitStack() as ph:
            px = ph.enter_context(tc.tile_pool(name="px", bufs=1))
            pgs = ph.enter_context(tc.tile_pool(name="pgs", bufs=1))
            pgp = ph.enter_context(tc.tile_pool(name="pgp", bufs=1, space="PSUM"))
            xs = [mktile(px, [P, L], f"x{ct}") for ct in range(CT)]
            # x streams on two DMA queues: ct0 via sync, ct1 via gpsimd
            for ct in range(CT):
                eng = nc.sync if ct == 0 else nc.gpsimd
                for lc in range(LC):
                    sl = slice(lc * 512, (lc + 1) * 512)
                    eng.dma_start(out=r(xs[ct][:, sl]),
                                  in_=r(x_ext[ct * P:(ct + 1) * P, sl]))
                    # bf16 copy for the QKV contraction, on ScalarE
                    nc.scalar.activation(x16[ct][:, sl], xs[ct][:, sl], ACT.Copy)
            # everything else follows x on the two queues
            for ct in range(CT):
                nc.sync.dma_start(out=smalls[ct][:],
                                  in_=smalls_ext[ct * P:(ct + 1) * P, :])
                nc.sync.dma_start(
                    out=r(indt_t[ct][:]),
                    in_=r(smalls_ext[ct * P:(ct + 1) * P, :]))
            nc.sync.dma_start(out=indTt[:], in_=indT_ext[:])
            for nm, ext in wts_dma:
                for ct in range(CT):
                    nc.gpsimd.dma_start(out=wts[nm][ct][:],
                                        in_=ext[ct * P:(ct + 1) * P, :])
            for h in range(NH):
                nc.sync.dma_start(out=wpth[h][:],
                                  in_=wp_ext[h * HD:(h + 1) * HD, :])
            # per-partition stats via bn_stats/bn_aggr, then a tiny PE
            # matmul with the group indicator to combine across partitions
            pm = []
            for ct in range(CT):
                st6 = mktile(pgs, [P, LC * 6], f"st6_{ct}")
                for lc in range(LC):
                    nc.vector.bn_stats(st6[:, lc * 6:(lc + 1) * 6],
                                       xs[ct][:, lc * 512:(lc + 1) * 512])
                mv = mktile(pgs, [P, 2], f"mv{ct}")
                nc.vector.bn_aggr(mv[:], st6[:].rearrange("p (a b) -> p a b", b=6))
                m2 = mktile(pgs, [P, 2], f"m2_{ct}")   # [mean, var + mean^2]
                nc.vector.tensor_copy(out=r(m2[:, 0:1]), in_=mv[:, 0:1])
                nc.vector.scalar_tensor_tensor(
                    out=r(m2[:, 1:2]), in0=mv[:, 0:1], scalar=0.0,
                    in1=mv[:, 0:1], op0=OP.add, op1=OP.mult)
                nc.vector.tensor_add(r(m2[:, 1:2]), r(m2[:, 1:2]).bitcast(f32),
                                     mv[:, 1:2])
                pm.append(m2)
            gsum = mktile(pgp, [G, 2], "gsum")
            for ct in range(CT):
                nc.tensor.matmul(gsum[:], r(indt[ct]), r(pm[ct][:]),
                                 start=(ct == 0), stop=(ct == CT - 1))
            inv_np = 1.0 / float(P // G * CT)
            mrs = mktile(pgs, [G, 2], "mrs")      # col0 = mean, col1 = rstd
            var = mktile(pgs, [G, 1], "var")
            sqv = mktile(pgs, [G, 1], "sqv")
            nc.vector.tensor_scalar_mul(mrs[:, 0:1], gsum[:, 0:1], inv_np)
            nc.vector.tensor_scalar_mul(var[:], gsum[:, 1:2], inv_np)
            nc.vector.tensor_mul(sqv[:], mrs[:, 0:1], mrs[:, 0:1])
            nc.vector.tensor_sub(var[:], var[:], sqv[:])
            eps_t = mktile(pgs, [G, 1], "eps")
            nc.vector.memset(eps_t[:], EPS)
            nc.scalar.activation(sqv[:], var[:], ACT.Ln, bias=eps_t[:])
            nc.scalar.activation(mrs[:, 1:2], sqv[:], ACT.Exp, scale=-0.5)
            # broadcast group stats to channels via PE: bc[c, :] = mrs[g(c), :]
            sts = []
            for ct in range(CT):
                bc_ps = mktile(pgp, [P, 2], f"bcps{ct}")
                nc.tensor.matmul(bc_ps[:], indTt[:, ct * P:(ct + 1) * P],
                                 mrs[:], start=True, stop=True)
                s_t = mktile(pgs, [P, 1], f"s{ct}")
                t_t = mktile(pgs, [P, 1], f"t{ct}")
                nc.vector.tensor_mul(s_t[:], bc_ps[:, 1:2], bias["gnw"][ct])
                nc.vector.tensor_mul(t_t[:], bc_ps[:, 0:1], s_t[:])
                nc.vector.tensor_sub(t_t[:], bias["gnb"][ct], t_t[:])
                sts.append((s_t, t_t))
            # fold GN into QKV: b' = W^T t + b (PE matvecs on the original
            # weights), then W' = W * diag(s) in place
            t16 = []
            for ct in range(CT):
                t = mktile(pgs, [P, 1], f"t16_{ct}", dtype=bf16)
                nc.vector.tensor_copy(out=t[:], in_=sts[ct][1][:])
                t16.append(t)
            for wnm, bnm in (("wqt", "bq"), ("wkt", "bk"), ("wvt", "bv")):
                for oct in range(CT):
                    osl = slice(oct * P, (oct + 1) * P)
                    fps = mktile(pgp, [P, 1], "fps")
                    for ct in range(CT):
                        nc.tensor.matmul(fps[:], wts[wnm][ct][:, osl],
                                         t16[ct][:],
                                         start=(ct == 0), stop=(ct == CT - 1))
                    nc.vector.tensor_add(bias[bnm][oct],
                                         bias[bnm][oct], fps[:])
            for wnm in ("wqt", "wkt", "wvt"):
                for ct in range(CT):
                    nc.vector.tensor_scalar_mul(wts[wnm][ct][:],
                                                wts[wnm][ct][:], sts[ct][0][:])

        # ---- pool C: q/k/VT (live through attention) -----------------
        pc = top.enter_context(tc.tile_pool(name="pc", bufs=1))
        qt = [mktile(pc, [P, DH], f"q{p}", dtype=bf16) for p in range(CT)]
        kt = [mktile(pc, [P, L], f"k{p}", dtype=bf16) for p in range(CT)]
        vt = [mktile(pc, [P, 65 * ECH], f"vt{h}", dtype=bf16) for h in range(NH)]

        # ---- QKV + V^T, as chunk closures ---------------------------
        vpair = [mktile(pc, [P, L], f"v{p}", dtype=bf16) for p in range(CT)]

        def qkv_chunks(p, paux):
            """Returns (early, deferred): minimal set needed for the first
            attention pairs, and the rest ordered by need."""
            osl = slice(p * P, (p + 1) * P)

            def qkv_go(wnm, bnm, dst, cchunk):
                def go():
                    sl = slice(cchunk * 512, (cchunk + 1) * 512)
                    ps = paux.tile([P, 512], f32, name="aux", tag="aux")
                    for ct in range(CT):
                        nc.tensor.matmul(ps[:], wts[wnm][ct][:, osl],
                                         x16[ct][:, sl],
                                         start=(ct == 0), stop=(ct == CT - 1))
                    nc.vector.tensor_scalar_add(dst[:, sl], ps[:], bias[bnm][p])
                return go

            def vt_go(ecg):
                def go():
                    pst = (paux.tile([P, 512], f32, name="aux", tag="aux")[:]
                           .bitcast(bf16)[:, 0:512])
                    for j in range(4):
                        esl = slice((ecg * 4 + j) * P, (ecg * 4 + j + 1) * P)
                        nc.tensor.transpose(pst[:, j * P:(j + 1) * P],
                                            vpair[p][:, esl], ident16[:])
                    for h01 in range(2):
                        head = 2 * p + h01
                        outap = (vt[head][:, ecg * 260:(ecg + 1) * 260]
                                 .rearrange("p (a b) -> p a b", b=65)[:, :, 0:64])
                        inap = (pst[:].rearrange("p (a b) -> p a b", b=P)
                                [:, :, h01 * 64:(h01 + 1) * 64])
                        nc.vector.tensor_copy(out=outap, in_=inap)
                return go

            def ones_go(h01):
                def go():
                    h = 2 * p + h01
                    ones_ap = (vt[h][:].rearrange("p (a b) -> p a b", b=65)
                               [:, :, 64:65])
                    nc.vector.tensor_copy(
                        out=ones_ap,
                        in_=ones_src[:, p * ECH:(p + 1) * ECH]
                        .rearrange("p a -> p a ()"))
                return go

            qw = [qkv_go("wqt", "bq", qt[p], c) for c in range(DBLK)]
            kw = [qkv_go("wkt", "bk", kt[p], c) for c in range(LC)]
            vw = [qkv_go("wvt", "bv", vpair[p], c) for c in range(LC)]
            tw = [vt_go(g) for g in range(NGRP)]
            ow = [ones_go(h01) for h01 in range(2)]
            early = [qw[0], kw[0], kw[1], vw[0], tw[0]] + ow
            deferred = [vw[1], tw[1]]
            for c in range(2, LC):
                deferred += [kw[c], vw[c], tw[c]]
            deferred += qw[1:]
            return early, deferred

        # ---- attention + per-db projection --------------------------
        with ExitStack() as ph:
            pe_s = ph.enter_context(tc.tile_pool(name="pes", bufs=3))
            pf = ph.enter_context(tc.tile_pool(name="pf", bufs=1))
            pfs = ph.enter_context(tc.tile_pool(name="pfs", bufs=3))
            ppt = ph.enter_context(tc.tile_pool(name="ppt", bufs=4))
            psp = ph.enter_context(tc.tile_pool(name="psp", bufs=1, space="PSUM"))
            pap = ph.enter_context(tc.tile_pool(name="pap", bufs=3, space="PSUM"))
            paux = ph.enter_context(tc.tile_pool(name="paux", bufs=1, space="PSUM"))
            xres = []
            for ct in range(CT):
                t = mktile(pf, [P, DH], f"xr{ct}")
                nc.sync.dma_start(out=t[:], in_=x_ext[ct * P:(ct + 1) * P, 0:DH])
                xres.append(t)
            e0, d0 = qkv_chunks(0, paux)
            e1, d1 = qkv_chunks(1, paux)
            for fn in e0 + e1[-2:]:
                fn()
            deferred = d0 + e1[:-2] + d1

            def epilogue_head(p, db, acc, h01):
                dsl = slice(db * 512, (db + 1) * 512)
                head = 2 * p + h01
                zc = mktile(pe_s, [65, 512], "zc", dtype=bf16)
                nc.scalar.activation(zc[:], acc[h01][0:65, :], ACT.Copy)
                zb_ps = paux.tile([P, 512], f32, name="aux", tag="aux")
                nc.tensor.matmul(zb_ps[0:64, :], bones16[64:65, 0:64],
                                 zc[64:65, :], start=True, stop=True)
                zb = mktile(pe_s, [64, 512], "zb")
                with nc.allow_low_precision("softmax denom, well conditioned"):
                    nc.vector.reciprocal_approx_fast(zb[:], zb_ps[0:64, :])
                nc.vector.tensor_mul(ao[head][:, dsl],
                                     zc[0:64, :], zb[:])

            def proj_ot(db, ot):
                dsl = slice(db * 512, (db + 1) * 512)
                osl = slice(ot * P, (ot + 1) * P)
                ps = paux.tile([P, 512], f32, name="aux", tag="aux")
                for h in range(NH):
                    nc.tensor.matmul(ps[:], wpth[h][:, osl],
                                     ao[h][:, dsl],
                                     start=(h == 0), stop=(h == NH - 1))
                osb = mktile(pfs, [P, 512], "osb")
                nc.vector.scalar_tensor_tensor(
                    out=osb[:], in0=ps[:], scalar=bias["bp"][ot],
                    in1=xres[ot][:, dsl], op0=OP.add, op1=OP.add)
                nc.sync.dma_start(out=out_ext[osl, dsl], in_=osb[:])

            def mkslots():
                return [[mktile(psp, [P, 512], f"sps{h01}{kti}")
                         for kti in range(2)] for h01 in range(2)]

            def s_quartet(p, db, pair, slots):
                # 4 S matmuls issued adjacently, one [128,512] psum tile
                # per (head, kti); head0 at PE rows 0-63, head1 at rows
                # 64-127 (row groups auto-derived from base partition) so
                # each kti's two matmuls overlap on the PE.
                dsl = slice(db * 512, (db + 1) * 512)
                for kti in range(2):
                    ec = 2 * pair + kti
                    esl = slice(ec * P, (ec + 1) * P)
                    for h01 in range(2):
                        hsl = slice(h01 * 64, (h01 + 1) * 64)
                        nc.tensor.matmul(
                            slots[h01][kti][:],
                            kt[p][hsl, esl], qt[p][hsl, dsl],
                            start=True, stop=True)

            pending = []
            for db in range(DBLK):
                for p in range(CT):
                    acc = [mktile(pap, [65, 512], "acc") for _ in range(2)]
                    slots = mkslots()
                    s_quartet(p, db, 0, slots)
                    for pair in range(NPAIR):
                        pts = [mktile(ppt, [P, 1024], "pt", dtype=u16)
                               for _ in range(2)]
                        for kti in range(2):
                            if pair >= 1 or db + p > 0:
                                budget = 2 if len(deferred) > 20 else 1
                                for _ in range(budget):
                                    if deferred:
                                        deferred.pop(0)()
                                    elif pending:
                                        pending.pop(0)()
                                        break
                            ksl = slice(kti * 512, (kti + 1) * 512)
                            # h0 on ScalarE (exact exp), h1 on VectorE
                            # (Schraudolph bits), per kti half so the two
                            # engines pipeline against the S quartets
                            nc.scalar.activation(
                                pts[0][:, ksl].bitcast(bf16),
                                slots[0][kti][:], ACT.Exp, scale=SCALE)
                            nc.vector.tensor_scalar(
                                pts[1][:, ksl], slots[1][kti][:],
                                DVE_A, DVE_B, op0=OP.mult, op1=OP.add)
                        if pair + 1 < NPAIR:
                            nslots = mkslots()
                            # tiny same-engine join writes: ScalarE stamps
                            # h1's next slots (so h1 S-matmuls wait on the
                            # ScalarE exps), VectorE stamps h0's (so h0
                            # waits on the VectorE exps).  Each quartet
                            # matmul then becomes ready only when BOTH
                            # engines finished that kti's exps -> the pair
                            # stays adjacent and overlaps on the PE.
                            for kti in range(2):
                                ko = kti * 512
                                nc.scalar.activation(
                                    nslots[1][kti][0:1, 0:1],
                                    pts[0][0:1, ko:ko + 1].bitcast(bf16),
                                    ACT.Copy)
                                nc.vector.tensor_copy(
                                    out=nslots[0][kti][0:1, 0:1],
                                    in_=pts[1][0:1, ko:ko + 1].bitcast(bf16))
                            slots = nslots
                            s_quartet(p, db, pair + 1, slots)
                        for h01 in range(2):
                            head = 2 * p + h01
                            pt16 = pts[h01][:].bitcast(bf16)
                            for kti in range(2):
                                ec = 2 * pair + kti
                                nc.tensor.matmul(
                                    acc[h01][:],
                                    vt[head][:, ec * 65:(ec + 1) * 65],
                                    pt16[:, kti * 512:(kti + 1) * 512],
                                    start=(pair + kti == 0),
                                    stop=(pair == NPAIR - 1 and kti == 1))
                    for h01 in range(2):
                        pending.append(
                            lambda p=p, db=db, acc=acc, h01=h01:
                            epilogue_head(p, db, acc, h01))
                    if p == CT - 1:
                        for ot in range(CT):
                            pending.append(lambda db=db, ot=ot: proj_ot(db, ot))
            for fn in deferred + pending:
                fn()

    nc.compile()
    return nc


def _in_maps(inputs):
    import ml_dtypes
    bf = ml_dtypes.bfloat16
    x = np.asarray(inputs["x"], dtype=np.float32)
    gnw = np.ascontiguousarray(np.asarray(inputs["gn_w"], np.float32).reshape(C, 1))
    gnb = np.ascontiguousarray(np.asarray(inputs["gn_b"], np.float32).reshape(C, 1))
    wqt = np.ascontiguousarray(np.asarray(inputs["wq"], np.float32).T.astype(bf))
    wkt = np.ascontiguousarray(np.asarray(inputs["wk"], np.float32).T.astype(bf))
    wvt = np.ascontiguousarray(np.asarray(inputs["wv"], np.float32).T.astype(bf))
    wpt = np.ascontiguousarray(np.asarray(inputs["wp"], np.float32).T.astype(bf))
    ind = np.zeros((C, G), np.float32)
    ind[np.arange(C), np.arange(C) // (C // G)] = 1.0
    indT = np.ascontiguousarray(ind.T)
    smalls = np.zeros((C, 16), np.float32)
    for j, nm in enumerate(("bq", "bk", "bv", "bp")):
        smalls[:, j] = np.asarray(inputs[nm], np.float32).reshape(C)
    smalls[:, 4] = gnw.reshape(C)
    smalls[:, 5] = gnb.reshape(C)
    smalls[:, 6:6 + G] = ind
    common = dict(wqt=wqt, wkt=wkt, wvt=wvt, wpt=wpt, smalls=smalls,
                  indT=indT)
    maps = []
    for core in range(NCORES):
        b, half = core // 2, core % 2
        xb = np.ascontiguousarray(x[b].reshape(C, L))
        if half == 1:
            xb = np.ascontiguousarray(
                np.concatenate([xb[:, DH:], xb[:, :DH]], axis=1))
        maps.append(dict(common, x=xb))
    return maps


def kernel(**inputs) -> np.ndarray:
    from concourse.bass_utils import run_bass_kernel_spmd

    if "nc" not in _CACHE:
        _CACHE["nc"] = _build_nc()
    nc = _CACHE["nc"]
    res = run_bass_kernel_spmd(nc, _in_maps(inputs), core_ids=list(range(NCORES)))
    out = np.empty((B, C, L), np.float32)
    for core in range(NCORES):
        b, half = core // 2, core % 2
        out[b][:, half * DH:(half + 1) * DH] = res.results[core]["out"]
    return out.reshape(B, C, L).reshape(B, C, H, W)


# revision 27
# speedup vs baseline: 1.3874x; 1.0165x over previous
"""Trainium2 Bass kernel for an AttentionBlock:
GroupNorm(8 groups) -> 1x1 conv q/k/v -> multi-head attention (4 heads)
-> 1x1 conv proj -> residual add.

Shapes (hardcoded): x [4, 256, 64, 64]; L = 64*64 = 4096; head dim 64.

Sharding: 8 cores = (batch, query-half). Each core computes the full
GroupNorm + K/V for its batch, and attention + projection + residual for
its half (2048) of the query positions. Host permutes each batch's pixel
columns so a core's query half is always columns 0:2048, so all 8 cores
run one SPMD program. No collectives; host concatenates.

v3 design (over the v2 bf16 baseline):
- S matmuls contract only K=64 (head dim), so the two heads of a channel
  tile are issued as an adjacent quartet at PE row groups 0-1 / 2-3
  (tile_position auto-derived from base partition 0 / 64): the PE runs
  the two heads' S matmuls concurrently, halving S cost.
- exp is split per head: head-even on ScalarE (exact exp -> bf16),
  head-odd on VectorE (Schraudolph: bf16 bit pattern via one
  tensor_scalar u16 = floor(a*S + b)), so both exps of a pair run
  concurrently and the S quartet of the next pair issues sooner.
- GroupNorm scale/shift is folded into the QKV weights on device
  (W' = W*diag(s), b' = W^T t + b via 12 tiny PE matvecs), so no
  normalized-activation pass exists; QKV contracts a bf16 copy of x
  made by ScalarE Copy during the x DMA window.
- A warm-up stream of dummy N=512 matmuls keeps the PE HAM clock gate
  at 8/8 (2.4 GHz) through the DMA/stats prologue; without it the PE
  runs at 1.2 GHz until ~56us in.
- softmax denominator Z via a 65th ones-column in V^T so the PV matmul
  accumulates Z as row 64; softmax max-subtraction skipped (logits in
  [-8, 8] for normalized inputs).
"""

import numpy as np

B, C, H, W = 4, 256, 64, 64
NH, G, EPS = 4, 8, 1e-5
L = H * W            # 4096
DH = L // 2          # query positions per core
HD = C // NH         # 64
P = 128              # SBUF partitions
CT = C // P          # channel tiles (2)
LC = L // 512        # 8 key-dim 512-chunks
DBLK = DH // 512     # 4 query-dim 512-blocks
ECH = L // P         # 32 key-dim 128-chunks
NPAIR = ECH // 2     # 16 key-dim 256-pairs
NGRP = ECH // 4      # 8 vT groups (4 chunks each = 2 pairs)
SCALE = float(HD) ** -0.5
NCORES = 8

LN2 = float(np.log(2.0))
DVE_A = 128.0 / LN2 * SCALE    # bf16 bits per raw logit (Schraudolph)
DVE_B = 128.0 * 127.0 + 0.5 - 5.5
WARMN = 32                     # HAM warm-up matmuls covering the prologue

_CACHE = {}


def _build_nc():
    import concourse.bacc as bacc
    import concourse.bass as bass
    import concourse.mybir as mybir
    import concourse.tile as tile
    from concourse.masks import make_identity
    from contextlib import ExitStack

    f32 = mybir.dt.float32
    f16 = mybir.dt.float16
    bf16 = mybir.dt.bfloat16
    f32r = mybir.dt.float32r
    u8 = mybir.dt.uint8
    u16 = mybir.dt.uint16
    AX = mybir.AxisListType
    OP = mybir.AluOpType
    ACT = mybir.ActivationFunctionType

    def r(ap):
        return ap.bitcast(f32r)

    def mktile(pool, shape, tag, dtype=None):
        return pool.tile(shape, dtype or f32, name=tag, tag=tag)

    nc = bacc.Bacc(trn_type="TRN2", target_bir_lowering=False, num_devices=NCORES)

    x_ext = nc.declare_dram_parameter("x", [C, L], f32, isOutput=False)
    wq_ext = nc.declare_dram_parameter("wqt", [C, C], bf16, isOutput=False)
    wk_ext = nc.declare_dram_parameter("wkt", [C, C], bf16, isOutput=False)
    wv_ext = nc.declare_dram_parameter("wvt", [C, C], bf16, isOutput=False)
    wp_ext = nc.declare_dram_parameter("wpt", [C, C], bf16, isOutput=False)
    # smalls[:, 0:6] = bq,bk,bv,bp,gnw,gnb; [:, 6:14] = group indicator
    smalls_ext = nc.declare_dram_parameter("smalls", [C, 16], f32, isOutput=False)
    indT_ext = nc.declare_dram_parameter("indT", [G, C], f32, isOutput=False)
    out_ext = nc.declare_dram_parameter("out", [C, DH], f32, isOutput=True)

    with tile.TileContext(nc) as tc, ExitStack() as top:
        # ---- pool A: kernel-long tiles -------------------------------
        pa = top.enter_context(tc.tile_pool(name="pa", bufs=1))
        warm16 = mktile(pa, [P, 512], "warm16", dtype=bf16)
        nc.vector.memset(warm16[:], 0.001)
        # allocate kernel-long tiles now; their DMAs are emitted AFTER the
        # x chunks so x owns the head of both DMA queues
        smalls = [mktile(pa, [P, 16], f"smalls{ct}") for ct in range(CT)]
        indTt = mktile(pa, [G, C], "indTt")
        indt_t = [mktile(pa, [P, 16], f"ind{ct}") for ct in range(CT)]
        bias = {}
        for j, nm in enumerate(("bq", "bk", "bv", "bp", "gnw", "gnb")):
            bias[nm] = [smalls[ct][:, j:j + 1] for ct in range(CT)]
        indt = [t[:, 6:6 + G] for t in indt_t]
        ones_t = mktile(pa, [P, 2 * ECH], "ones_t")
        nc.vector.memset(ones_t[:], 1.0)
        ones_src = ones_t[:]
        wts = {}
        for nm, ext in (("wqt", wq_ext), ("wkt", wk_ext), ("wvt", wv_ext)):
            wts[nm] = []
            for ct in range(CT):
                t = mktile(pa, [P, C], f"{nm}{ct}", dtype=bf16)
                wts[nm].append(t)
        wts_dma = [(nm, ext) for nm, ext in
                   (("wqt", wq_ext), ("wkt", wk_ext), ("wvt", wv_ext))]
        wpth = [mktile(pa, [HD, C], f"wpt{h}", dtype=bf16) for h in range(NH)]
        ident = mktile(pa, [P, P], "ident")
        make_identity(nc, ident[:])
        ident16 = mktile(pa, [P, P], "ident16", dtype=bf16)
        nc.vector.tensor_copy(out=ident16[:], in_=ident[:])
        bones16 = mktile(pa, [P, 64], "bones16", dtype=bf16)
        nc.vector.memset(bones16[:], 1.0)
        # attention output, one tile per head [64, DH] at base partition 0
        ao = [mktile(pa, [HD, DH], f"ao{h}", dtype=bf16) for h in range(NH)]

        # ---- pool B: bf16 copy of x (QKV moving operand) -------------
        pb = top.enter_context(tc.tile_pool(name="pb", bufs=1))
        x16 = [mktile(pb, [P, L], f"x16_{ct}", dtype=bf16) for ct in range(CT)]

        # ---- warm-up: HAM stream + activation tables -----------------
        with ExitStack() as pw:
            pwp = pw.enter_context(tc.tile_pool(name="pwp", bufs=1, space="PSUM"))
            pws = pw.enter_context(tc.tile_pool(name="pws", bufs=1))
            wps = mktile(pwp, [P, 512], "wps")
            for i in range(WARMN):
                nc.tensor.matmul(wps[:], warm16[:, 0:P], warm16[:],
                                 start=True, stop=True)
            wsc = mktile(pws, [P, 2], "wsc")
            nc.scalar.activation(wsc[:, 0:1], warm16[:, 0:1].bitcast(bf16), ACT.Exp)
            nc.scalar.activation(wsc[:, 1:2], wsc[:, 0:1], ACT.Ln)

        # ---- GroupNorm stats + weight fold ---------------------------
        with ExitStack() as ph:
            px = ph.enter_context(tc.tile_pool(name="px", bufs=1))
            pgs = ph.enter_context(tc.tile_pool(name="pgs", bufs=1))
            pgp = ph.enter_context(tc.tile_pool(name="pgp", bufs=1, space="PSUM"))
            xs = [mktile(px, [P, L], f"x{ct}") for ct in range(CT)]
            # x streams on two DMA queues: ct0 via sync, ct1 via gpsimd
            for ct in range(CT):
                eng = nc.sync if ct == 0 else nc.gpsimd
                for lc in range(LC):
                    sl = slice(lc * 512, (lc + 1) * 512)
                    eng.dma_start(out=r(xs[ct][:, sl]),
                                  in_=r(x_ext[ct * P:(ct + 1) * P, sl]))
                    # bf16 copy for the QKV contraction, on ScalarE
                    nc.scalar.activation(x16[ct][:, sl], xs[ct][:, sl], ACT.Copy)
            # everything else follows x on the two queues
            for ct in range(CT):
                nc.sync.dma_start(out=smalls[ct][:],
                                  in_=smalls_ext[ct * P:(ct + 1) * P, :])
                nc.sync.dma_start(
                    out=r(indt_t[ct][:]),
                    in_=r(smalls_ext[ct * P:(ct + 1) * P, :]))
            nc.sync.dma_start(out=indTt[:], in_=indT_ext[:])
            for nm, ext in wts_dma:
                for ct in range(CT):
                    nc.gpsimd.dma_start(out=wts[nm][ct][:],
                                        in_=ext[ct * P:(ct + 1) * P, :])
            for h in range(NH):
                nc.sync.dma_start(out=wpth[h][:],
                                  in_=wp_ext[h * HD:(h + 1) * HD, :])
            # per-partition stats via bn_stats/bn_aggr, then a tiny PE
            # matmul with the group indicator to combine across partitions
            pm = []
            for ct in range(CT):
                st6 = mktile(pgs, [P, LC * 6], f"st6_{ct}")
                for lc in range(LC):
                    nc.vector.bn_stats(st6[:, lc * 6:(lc + 1) * 6],
                                       xs[ct][:, lc * 512:(lc + 1) * 512])
                mv = mktile(pgs, [P, 2], f"mv{ct}")
                nc.vector.bn_aggr(mv[:], st6[:].rearrange("p (a b) -> p a b", b=6))
                m2 = mktile(pgs, [P, 2], f"m2_{ct}")   # [mean, var + mean^2]
                nc.vector.tensor_copy(out=r(m2[:, 0:1]), in_=mv[:, 0:1])
                nc.vector.scalar_tensor_tensor(
                    out=r(m2[:, 1:2]), in0=mv[:, 0:1], scalar=0.0,
                    in1=mv[:, 0:1], op0=OP.add, op1=OP.mult)
                nc.vector.tensor_add(r(m2[:, 1:2]), r(m2[:, 1:2]).bitcast(f32),
                                     mv[:, 1:2])
                pm.append(m2)
            gsum = mktile(pgp, [G, 2], "gsum")
            for ct in range(CT):
                nc.tensor.matmul(gsum[:], r(indt[ct]), r(pm[ct][:]),
                                 start=(ct == 0), stop=(ct == CT - 1))
            inv_np = 1.0 / float(P // G * CT)
            mrs = mktile(pgs, [G, 2], "mrs")      # col0 = mean, col1 = rstd
            var = mktile(pgs, [G, 1], "var")
            sqv = mktile(pgs, [G, 1], "sqv")
            nc.vector.tensor_scalar_mul(mrs[:, 0:1], gsum[:, 0:1], inv_np)
            nc.vector.tensor_scalar_mul(var[:], gsum[:, 1:2], inv_np)
            nc.vector.tensor_mul(sqv[:], mrs[:, 0:1], mrs[:, 0:1])
            nc.vector.tensor_sub(var[:], var[:], sqv[:])
            eps_t = mktile(pgs, [G, 1], "eps")
            nc.vector.memset(eps_t[:], EPS)
            nc.scalar.activation(sqv[:], var[:], ACT.Ln, bias=eps_t[:])
            nc.scalar.activation(mrs[:, 1:2], sqv[:], ACT.Exp, scale=-0.5)
            # broadcast group stats to channels via PE: bc[c, :] = mrs[g(c), :]
            sts = []
            for ct in range(CT):
                bc_ps = mktile(pgp, [P, 2], f"bcps{ct}")
                nc.tensor.matmul(bc_ps[:], indTt[:, ct * P:(ct + 1) * P],
                                 mrs[:], start=True, stop=True)
                s_t = mktile(pgs, [P, 1], f"s{ct}")
                t_t = mktile(pgs, [P, 1], f"t{ct}")
                nc.vector.tensor_mul(s_t[:], bc_ps[:, 1:2], bias["gnw"][ct])
                nc.vector.tensor_mul(t_t[:], bc_ps[:, 0:1], s_t[:])
                nc.vector.tensor_sub(t_t[:], bias["gnb"][ct], t_t[:])
                sts.append((s_t, t_t))
            # fold GN into QKV: b' = W^T t + b (PE matvecs on the original
            # weights), then W' = W * diag(s) in place
            t16 = []
            for ct in range(CT):
                t = mktile(pgs, [P, 1], f"t16_{ct}", dtype=bf16)
                nc.vector.tensor_copy(out=t[:], in_=sts[ct][1][:])
                t16.append(t)
            for wnm, bnm in (("wqt", "bq"), ("wkt", "bk"), ("wvt", "bv")):
                for oct in range(CT):
                    osl = slice(oct * P, (oct + 1) * P)
                    fps = mktile(pgp, [P, 1], "fps")
                    for ct in range(CT):
                        nc.tensor.matmul(fps[:], wts[wnm][ct][:, osl],
                                         t16[ct][:],
                                         start=(ct == 0), stop=(ct == CT - 1))
                    nc.vector.tensor_add(bias[bnm][oct],
                                         bias[bnm][oct], fps[:])
            for wnm in ("wqt", "wkt", "wvt"):
                for ct in range(CT):
                    nc.vector.tensor_scalar_mul(wts[wnm][ct][:],
                                                wts[wnm][ct][:], sts[ct][0][:])

        # ---- pool C: q/k/VT (live through attention) -----------------
        pc = top.enter_context(tc.tile_pool(name="pc", bufs=1))
        qt = [mktile(pc, [P, DH], f"q{p}", dtype=bf16) for p in range(CT)]
        kt = [mktile(pc, [P, L], f"k{p}", dtype=bf16) for p in range(CT)]
        vt = [mktile(pc, [P, 65 * ECH], f"vt{h}", dtype=bf16) for h in range(NH)]

        # ---- QKV + V^T, as chunk closures ---------------------------
        vpair = [mktile(pc, [P, L], f"v{p}", dtype=bf16) for p in range(CT)]

        def qkv_chunks(p, paux):
            """Returns (early, deferred): minimal set needed for the first
            attention pairs, and the rest ordered by need."""
            osl = slice(p * P, (p + 1) * P)

            def qkv_go(wnm, bnm, dst, cchunk):
                def go():
                    sl = slice(cchunk * 512, (cchunk + 1) * 512)
                    ps = paux.tile([P, 512], f32, name="aux", tag="aux")
                    for ct in range(CT):
                        nc.tensor.matmul(ps[:], wts[wnm][ct][:, osl],
                                         x16[ct][:, sl],
                                         start=(ct == 0), stop=(ct == CT - 1))
                    nc.vector.tensor_scalar_add(dst[:, sl], ps[:], bias[bnm][p])
                return go

            def vt_go(ecg):
                def go():
                    pst = (paux.tile([P, 512], f32, name="aux", tag="aux")[:]
                           .bitcast(bf16)[:, 0:512])
                    for j in range(4):
                        esl = slice((ecg * 4 + j) * P, (ecg * 4 + j + 1) * P)
                        nc.tensor.transpose(pst[:, j * P:(j + 1) * P],
                                            vpair[p][:, esl], ident16[:])
                    for h01 in range(2):
                        head = 2 * p + h01
                        outap = (vt[head][:, ecg * 260:(ecg + 1) * 260]
                                 .rearrange("p (a b) -> p a b", b=65)[:, :, 0:64])
                        inap = (pst[:].rearrange("p (a b) -> p a b", b=P)
                                [:, :, h01 * 64:(h01 + 1) * 64])
                        nc.vector.tensor_copy(out=outap, in_=inap)
                return go

            def ones_go(h01):
                def go():
                    h = 2 * p + h01
                    ones_ap = (vt[h][:].rearrange("p (a b) -> p a b", b=65)
                               [:, :, 64:65])
                    nc.vector.tensor_copy(
                        out=ones_ap,
                        in_=ones_src[:, p * ECH:(p + 1) * ECH]
                        .rearrange("p a -> p a ()"))
                return go

            qw = [qkv_go("wqt", "bq", qt[p], c) for c in range(DBLK)]
            kw = [qkv_go("wkt", "bk", kt[p], c) for c in range(LC)]
            vw = [qkv_go("wvt", "bv", vpair[p], c) for c in range(LC)]
            tw = [vt_go(g) for g in range(NGRP)]
            ow = [ones_go(h01) for h01 in range(2)]
            early = [qw[0], kw[0], kw[1], vw[0], tw[0]] + ow
            deferred = [vw[1], tw[1]]
            for c in range(2, LC):
                deferred += [kw[c], vw[c], tw[c]]
            deferred += qw[1:]
            return early, deferred

        # ---- attention + per-db projection --------------------------
        with ExitStack() as ph:
            pe_s = ph.enter_context(tc.tile_pool(name="pes", bufs=3))
            pf = ph.enter_context(tc.tile_pool(name="pf", bufs=1))
            pfs = ph.enter_context(tc.tile_pool(name="pfs", bufs=3))
            ppt = ph.enter_context(tc.tile_pool(name="ppt", bufs=4))
            psp = ph.enter_context(tc.tile_pool(name="psp", bufs=2, space="PSUM"))
            pap = ph.enter_context(tc.tile_pool(name="pap", bufs=3, space="PSUM"))
            paux = ph.enter_context(tc.tile_pool(name="paux", bufs=1, space="PSUM"))
            xres = []
            for ct in range(CT):
                t = mktile(pf, [P, DH], f"xr{ct}")
                nc.sync.dma_start(out=t[:], in_=x_ext[ct * P:(ct + 1) * P, 0:DH])
                xres.append(t)
            e0, d0 = qkv_chunks(0, paux)
            e1, d1 = qkv_chunks(1, paux)
            for fn in e0 + e1[-2:]:
                fn()
            deferred = d0 + e1[:-2] + d1

            def epilogue_head(p, db, acc, h01):
                dsl = slice(db * 512, (db + 1) * 512)
                head = 2 * p + h01
                zc = mktile(pe_s, [65, 512], "zc", dtype=bf16)
                nc.scalar.activation(zc[:], acc[h01][0:65, :], ACT.Copy)
                zb_ps = paux.tile([P, 512], f32, name="aux", tag="aux")
                nc.tensor.matmul(zb_ps[0:64, :], bones16[64:65, 0:64],
                                 zc[64:65, :], start=True, stop=True)
                zb = mktile(pe_s, [64, 512], "zb")
                with nc.allow_low_precision("softmax denom, well conditioned"):
                    nc.vector.reciprocal_approx_fast(zb[:], zb_ps[0:64, :])
                nc.vector.tensor_mul(ao[head][:, dsl],
                                     zc[0:64, :], zb[:])

            def proj_ot(db, ot):
                dsl = slice(db * 512, (db + 1) * 512)
                osl = slice(ot * P, (ot + 1) * P)
                ps = paux.tile([P, 512], f32, name="aux", tag="aux")
                for h in range(NH):
                    nc.tensor.matmul(ps[:], wpth[h][:, osl],
                                     ao[h][:, dsl],
                                     start=(h == 0), stop=(h == NH - 1))
                osb = mktile(pfs, [P, 512], "osb")
                nc.vector.scalar_tensor_tensor(
                    out=osb[:], in0=ps[:], scalar=bias["bp"][ot],
                    in1=xres[ot][:, dsl], op0=OP.add, op1=OP.add)
                nc.sync.dma_start(out=out_ext[osl, dsl], in_=osb[:])

            def s_quartet(p, db, pair, slots):
                # 4 S matmuls issued adjacently; head0 at PE rows 0-63,
                # head1 at rows 64-127 (row groups auto-derived from base
                # partition) so the two heads can overlap on the array.
                dsl = slice(db * 512, (db + 1) * 512)
                for kti in range(2):
                    ec = 2 * pair + kti
                    esl = slice(ec * P, (ec + 1) * P)
                    for h01 in range(2):
                        hsl = slice(h01 * 64, (h01 + 1) * 64)
                        nc.tensor.matmul(
                            slots[h01][:, kti * 512:(kti + 1) * 512],
                            kt[p][hsl, esl], qt[p][hsl, dsl],
                            start=True, stop=True)

            pending = []
            for db in range(DBLK):
                for p in range(CT):
                    acc = [mktile(pap, [65, 512], "acc") for _ in range(2)]
                    slots = [mktile(psp, [P, 1024], "sps") for _ in range(2)]
                    s_quartet(p, db, 0, slots)
                    for pair in range(NPAIR):
                        pts = []
                        for h01 in range(2):
                            if pair >= 1 or db + p > 0:
                                budget = 2 if len(deferred) > 20 else 1
                                for _ in range(budget):
                                    if deferred:
                                        deferred.pop(0)()
                                    elif pending:
                                        pending.pop(0)()
                                        break
                            pt = mktile(ppt, [P, 1024], "pt", dtype=u16)
                            if h01 == 1:
                                nc.vector.tensor_scalar(
                                    pt[:], slots[h01][:], DVE_A, DVE_B,
                                    op0=OP.mult, op1=OP.add)
                            else:
                                nc.scalar.activation(pt[:].bitcast(bf16),
                                                     slots[h01][:], ACT.Exp,
                                                     scale=SCALE)
                            pts.append(pt)
                        if pair + 1 < NPAIR:
                            slots = [mktile(psp, [P, 1024], "sps")
                                     for _ in range(2)]
                            s_quartet(p, db, pair + 1, slots)
                        for h01 in range(2):
                            head = 2 * p + h01
                            pt16 = pts[h01][:].bitcast(bf16)
                            for kti in range(2):
                                ec = 2 * pair + kti
                                nc.tensor.matmul(
                                    acc[h01][:],
                                    vt[head][:, ec * 65:(ec + 1) * 65],
                                    pt16[:, kti * 512:(kti + 1) * 512],
                                    start=(pair + kti == 0),
                                    stop=(pair == NPAIR - 1 and kti == 1))
                    for h01 in range(2):
                        pending.append(
                            lambda p=p, db=db, acc=acc, h01=h01:
                            epilogue_head(p, db, acc, h01))
                    if p == CT - 1:
                        for ot in range(CT):
                            pending.append(lambda db=db, ot=ot: proj_ot(db, ot))
            for fn in deferred + pending:
                fn()

    nc.compile()
    return nc


def _in_maps(inputs):
    import ml_dtypes
    bf = ml_dtypes.bfloat16
    x = np.asarray(inputs["x"], dtype=np.float32)
    gnw = np.ascontiguousarray(np.asarray(inputs["gn_w"], np.float32).reshape(C, 1))
    gnb = np.ascontiguousarray(np.asarray(inputs["gn_b"], np.float32).reshape(C, 1))
    wqt = np.ascontiguousarray(np.asarray(inputs["wq"], np.float32).T.astype(bf))
    wkt = np.ascontiguousarray(np.asarray(inputs["wk"], np.float32).T.astype(bf))
    wvt = np.ascontiguousarray(np.asarray(inputs["wv"], np.float32).T.astype(bf))
    wpt = np.ascontiguousarray(np.asarray(inputs["wp"], np.float32).T.astype(bf))
    ind = np.zeros((C, G), np.float32)
    ind[np.arange(C), np.arange(C) // (C // G)] = 1.0
    indT = np.ascontiguousarray(ind.T)
    smalls = np.zeros((C, 16), np.float32)
    for j, nm in enumerate(("bq", "bk", "bv", "bp")):
        smalls[:, j] = np.asarray(inputs[nm], np.float32).reshape(C)
    smalls[:, 4] = gnw.reshape(C)
    smalls[:, 5] = gnb.reshape(C)
    smalls[:, 6:6 + G] = ind
    common = dict(wqt=wqt, wkt=wkt, wvt=wvt, wpt=wpt, smalls=smalls,
                  indT=indT)
    maps = []
    for core in range(NCORES):
        b, half = core // 2, core % 2
        xb = np.ascontiguousarray(x[b].reshape(C, L))
        if half == 1:
            xb = np.ascontiguousarray(
                np.concatenate([xb[:, DH:], xb[:, :DH]], axis=1))
        maps.append(dict(common, x=xb))
    return maps


def kernel(**inputs) -> np.ndarray:
    from concourse.bass_utils import run_bass_kernel_spmd

    if "nc" not in _CACHE:
        _CACHE["nc"] = _build_nc()
    nc = _CACHE["nc"]
    res = run_bass_kernel_spmd(nc, _in_maps(inputs), core_ids=list(range(NCORES)))
    out = np.empty((B, C, L), np.float32)
    for core in range(NCORES):
        b, half = core // 2, core % 2
        out[b][:, half * DH:(half + 1) * DH] = res.results[core]["out"]
    return out.reshape(B, C, L).reshape(B, C, H, W)
